# revision 38
# baseline (speedup 1.0000x reference)
"""Trainium2 Bass kernel for nn_DMLoss_61942018343083 (Chamfer-style polygon
matching loss, retrieval_knn).

Sharding: data-parallel over batch B=32 across 8 NeuronCores (4 batches/core).
Each core computes partial sums into a [128, 12] output tile; the host combines
them into the scalar loss.

v3 design (exact-grid segment key in residual form):

pred2gt: for pred p and segment i (start v_i, dir s_i, 10 grid points at
  t/10), the exact grid-min key is
      key = -C^2 - ((z - t*) * L/10)^2,  t* = clamp(round(z), 0, 9)
  where C = perp line distance (linear form in p -> ONE K=17 bf16 monomial
  matmul gives -C^2 directly), z = 10(w.s)/len2 (K=8 bf16 matmul).
  t* via 3 scalar ACTs (Relu, 9-Relu, +1024 fp16 round trick: fp16 ulp at
  1024 is exactly 1, so the fp16 output write rounds c2 to an integer);
  the key via 4 DVE ops (residual STT reading psZ, x(L/10), square,
  -m^2+psNC STT). fp16 keys are safe because both -C^2 and the residual
  term are small near the argmax (no cancellation; far columns saturate to
  -inf which max/find handle fine). reduce(max) + find_index8 give the
  segment; a per-chunk indirect DMA (element_offset = per-batch bias into
  a per-core table) gathers the winning segment's 10 interp points; exact
  fp32 refine with tie-count normalization picks the true nearest.

gt2pred: K=10 bf16 matmul gives key2 = -|g-p|^2 directly (the -|g|^2 rows
  are folded into the matmul so values near the max are small ->
  fp16-safe, and no scalar bias ACT is needed); scalar Copy drains to
  fp16, reduce+find_index8 pick the nearest pred, indirect DMA gathers its
  pred_polys_ coords.

Empirical notes driving the design (from neuron-profile traces):
  - DVE TENSOR_REDUCE and FIND_INDEX8 run at 1 elem/cycle regardless of
    dtype here; TT/STT with all-SBUF 2-byte operands run 2x.
  - Indirect DMA costs ~994ns fixed + 0.34ns/descriptor on gpsimd; the 32
    per-chunk gathers (~36us busy) pipeline under the vector-bound main
    loop. Multi-offset (ap=[128,k>1]) indirect DMA mis-executes on HW
    (only the first offset column is honored) - do not batch gathers.
  - Slotted find_index8 (multiple chunk maxes in the 8 in_max slots over a
    concatenated in_values) is both slower and inaccurate on HW.
"""

import os
import sys

for _p in ("/opt/trn_rl_repo", "/root/.axon_site/_ro/trn_rl_repo"):
    if os.path.isdir(_p) and _p not in sys.path:
        sys.path.insert(0, _p)

import numpy as np
import ml_dtypes

bfloat16 = ml_dtypes.bfloat16

import concourse.bass as bass
import concourse.bacc as bacc
import concourse.mybir as mybir
from concourse.bass import IndirectOffsetOnAxis
from concourse.bass_utils import run_bass_kernel_spmd
from concourse.tile import TileContext

F32 = mybir.dt.float32
BF16 = mybir.dt.bfloat16
FP16 = mybir.dt.float16
U32 = mybir.dt.uint32
AF = mybir.ActivationFunctionType
ALU = mybir.AluOpType
AX = mybir.AxisListType

DEBUG = bool(int(os.environ.get("DMLOSS_DEBUG", "0")))

B, NP, NG, T = 32, 512, 512, 10
NCORES = 8
BLOC = B // NCORES          # 4 batches per core
NCH = NP // 128             # 4 chunks of 128 preds / 128 gts
CEN = np.float32(256.0)     # recentering shift
KZ = 8                      # z matmul contraction rows
KNC = 17                    # -C^2 monomial matmul rows
KG = 10                     # gt2pred contraction rows


def _split_hi_lo(x):
    x = np.asarray(x, dtype=np.float32)
    hi = x.astype(bfloat16)
    lo = (x - hi.astype(np.float32)).astype(bfloat16)
    return hi, lo


def host_prep(ini_pred_poly, pred_polys_, gt_polys):
    """Build all matmul operands / tables for one core's BLOC batches."""
    f = np.float32
    ini = np.asarray(ini_pred_poly, dtype=np.float32)   # [BLOC, NP, 2]
    p2 = np.asarray(pred_polys_, dtype=np.float32)      # [BLOC, NP, 2]
    gt = np.asarray(gt_polys, dtype=np.float32)         # [BLOC, NG, 2]
    v = np.roll(gt, 1, axis=1)
    s = (gt - v).astype(f)
    len2 = (s * s).sum(-1).astype(f)
    good = len2 > 1e-6
    L = np.sqrt(np.maximum(len2, 1e-12)).astype(f)
    inv = np.where(good, (f(10.0) / np.maximum(len2, f(1e-9))), f(0.0)).astype(f)

    pc = (ini - CEN).astype(f)
    vc = (v - CEN).astype(f)
    gc = (gt - CEN).astype(f)

    pxh, pxl = _split_hi_lo(pc[:, :, 0])
    pyh, pyl = _split_hi_lo(pc[:, :, 1])
    m1 = np.full_like(pxh, -1.0)
    one_p = np.ones_like(pxh)

    # ---- Z matmul: z = (p'.s - v'.s) * 10/len2 ----
    sx = (s[:, :, 0] * inv).astype(f)
    sy = (s[:, :, 1] * inv).astype(f)
    gs = ((vc * s).sum(-1) * inv).astype(f)
    sxh, sxl = _split_hi_lo(sx)
    syh, syl = _split_hi_lo(sy)
    gsh, gsl = _split_hi_lo(gs)
    lhsZ = np.stack([pxh, pxh, pxl, pyh, pyh, pyl, m1, m1],
                    axis=1).astype(bfloat16)                  # [BLOC, 8, NP]
    rhsZ = np.stack([sxh, sxl, sxh, syh, syl, syh, gsh, gsl],
                    axis=1).astype(bfloat16)                  # [BLOC, 8, NG]

    # ---- NC matmul: -C^2, C = a p'x + b p'y + c (perp line distance) ----
    with np.errstate(divide="ignore", invalid="ignore"):
        a = np.where(good, s[:, :, 1] / L, f(0.0)).astype(f)
        bco = np.where(good, -s[:, :, 0] / L, f(0.0)).astype(f)
    c0 = -(vc[:, :, 0] * a + vc[:, :, 1] * bco)
    # degenerate (zero-length) segments: kill their columns (covered by the
    # next segment's t=0 point)
    ck = np.where(good, c0, f(30000.0)).astype(f)

    X2 = (pc[:, :, 0] * pc[:, :, 0]).astype(f)
    XY = (pc[:, :, 0] * pc[:, :, 1]).astype(f)
    Y2 = (pc[:, :, 1] * pc[:, :, 1]).astype(f)
    A1 = (-(a * a)).astype(f)
    A2 = (-(2 * a * bco)).astype(f)
    A3 = (-(bco * bco)).astype(f)
    A4 = (-(2 * a * ck)).astype(f)
    A5 = (-(2 * bco * ck)).astype(f)
    A6 = (-(ck * ck)).astype(f)

    lhs_rows, rhs_rows = [], []
    for P, A in ((X2, A1), (XY, A2), (Y2, A3),
                 (pc[:, :, 0], A4), (pc[:, :, 1], A5)):
        Ph, Pl = _split_hi_lo(P)
        Ah, Al = _split_hi_lo(A)
        lhs_rows += [Ph, Ph, Pl]
        rhs_rows += [Ah, Al, Ah]
    A6h, A6l = _split_hi_lo(A6)
    lhs_rows += [one_p, one_p]
    rhs_rows += [A6h, A6l]
    lhsNC = np.stack(lhs_rows, axis=1).astype(bfloat16)       # [BLOC, 17, NP]
    rhsNC = np.stack(rhs_rows, axis=1).astype(bfloat16)       # [BLOC, 17, NG]

    # ---- gt2pred: key2 = 2g'.p' - |p'|^2 - |g'|^2 = -d^2 ----
    g2xh, g2xl = _split_hi_lo(f(2.0) * gc[:, :, 0])
    g2yh, g2yl = _split_hi_lo(f(2.0) * gc[:, :, 1])
    m1g = np.full_like(g2xh, -1.0)
    ngh, ngl = _split_hi_lo(-(gc * gc).sum(-1))
    gtl10 = np.stack([g2xh, g2xh, g2xl, g2yh, g2yh, g2yl, m1g, m1g, ngh, ngl],
                     axis=1).astype(bfloat16)                 # [BLOC, 10, NG]
    pp = (pc * pc).sum(-1).astype(f)
    pph, ppl = _split_hi_lo(pp)
    prhs10 = np.stack([pxh, pxl, pxh, pyh, pyl, pyh, pph, ppl, one_p, one_p],
                      axis=1).astype(bfloat16)                # [BLOC, 10, NP]

    # ---- slf: L/10 per column, replicated across partitions, fp16 ----
    sl = (L / f(10.0)).astype(np.float16)                     # [BLOC, NG]
    slf = np.broadcast_to(sl[:, None, :], (BLOC, 128, NG)).copy()

    # ---- interp table, bit-exact ref math: [BLOC*NG, T*2] f32 ----
    a_t = (np.arange(T, dtype=f) / f(T)).astype(f)
    itab = np.empty((BLOC, NG, T, 2), dtype=f)
    for t in range(T):
        itab[:, :, t, :] = (gt * a_t[t]).astype(f) + (v * (f(1.0) - a_t[t])).astype(f)
    itabAll = itab.reshape(BLOC * NG, T * 2)

    # ---- pred table for gt2pred gather: [BLOC*NP, 2] f32 ----
    ptabAll = p2.reshape(BLOC * NP, 2).astype(f)

    return dict(lhsZ=lhsZ, rhsZ=rhsZ, lhsNC=lhsNC, rhsNC=rhsNC,
                gtl10=gtl10, prhs10=prhs10, slf=slf,
                itabAll=itabAll, ptabAll=ptabAll)


def build_nc():
    nc = bacc.Bacc()

    ini = nc.dram_tensor("ini_pred_poly", [BLOC, NP, 2], F32, kind="ExternalInput")
    pred2 = nc.dram_tensor("pred_polys_", [BLOC, NP, 2], F32, kind="ExternalInput")
    gt = nc.dram_tensor("gt_polys", [BLOC, NG, 2], F32, kind="ExternalInput")
    kmask = nc.dram_tensor("keyPointsMask", [BLOC, NG], F32, kind="ExternalInput")
    lhsZ_d = nc.dram_tensor("lhsZ", [BLOC, KZ, NP], BF16, kind="ExternalInput")
    rhsZ_d = nc.dram_tensor("rhsZ", [BLOC, KZ, NG], BF16, kind="ExternalInput")
    lhsNC_d = nc.dram_tensor("lhsNC", [BLOC, KNC, NP], BF16, kind="ExternalInput")
    rhsNC_d = nc.dram_tensor("rhsNC", [BLOC, KNC, NG], BF16, kind="ExternalInput")
    gtl10_d = nc.dram_tensor("gtl10", [BLOC, KG, NG], BF16, kind="ExternalInput")
    prhs10_d = nc.dram_tensor("prhs10", [BLOC, KG, NP], BF16, kind="ExternalInput")
    slf_d = nc.dram_tensor("slf", [BLOC, 128, NG], FP16, kind="ExternalInput")
    itab_d = nc.dram_tensor("itabAll", [BLOC * NG, T * 2], F32, kind="ExternalInput")
    ptab_d = nc.dram_tensor("ptabAll", [BLOC * NP, 2], F32, kind="ExternalInput")
    out = nc.dram_tensor("out", [128, 12], F32, kind="ExternalOutput")
    if DEBUG:
        dbg_key = nc.dram_tensor("dbg_key", [128, NG], FP16, kind="ExternalOutput")
        dbg_pb = nc.dram_tensor("dbg_pb", [128, NP], FP16, kind="ExternalOutput")
        dbg_candC = nc.dram_tensor("dbg_candC", [128, BLOC * NCH * T * 2], F32, kind="ExternalOutput")

    NSL = BLOC * NCH  # 16 (batch, chunk) slots

    with TileContext(nc) as tc:
        with (
            tc.tile_pool(name="const", bufs=1) as cpool,
            tc.tile_pool(name="bat", bufs=2) as bat,
            tc.tile_pool(name="drp", bufs=5) as drp,
            tc.tile_pool(name="mrg", bufs=4) as mrg,
            tc.tile_pool(name="small", bufs=5) as small,
            tc.tile_pool(name="psZ", bufs=2, space="PSUM") as pszp,
            tc.tile_pool(name="psNC", bufs=4, space="PSUM") as psncp,
            tc.tile_pool(name="psG", bufs=2, space="PSUM") as psgp,
        ):
            res = cpool.tile([128, 12], F32)
            nc.vector.memset(res[:], 0.0)
            c9 = cpool.tile([128, 1], F32)
            nc.vector.memset(c9[:], 9.0)
            candC = cpool.tile([128, BLOC, NCH, T, 2], F32)
            npredC = cpool.tile([128, BLOC, NCH, 2], F32)
            pxyC = cpool.tile([128, BLOC, NCH, 2], F32)
            pred2C = cpool.tile([128, BLOC, NCH, 2], F32)
            gtC = cpool.tile([128, BLOC, NCH, 2], F32)
            maskC = cpool.tile([128, BLOC, NCH], F32)


            for b_ in range(BLOC):
                lhsZ = bat.tile([KZ, NP], BF16, tag="lhsZ")
                nc.sync.dma_start(out=lhsZ[:], in_=lhsZ_d[b_])
                rhsZ = bat.tile([KZ, NG], BF16, tag="rhsZ")
                nc.sync.dma_start(out=rhsZ[:], in_=rhsZ_d[b_])
                lhsNC = bat.tile([KNC, NP], BF16, tag="lhsNC")
                nc.scalar.dma_start(out=lhsNC[:], in_=lhsNC_d[b_])
                rhsNC = bat.tile([KNC, NG], BF16, tag="rhsNC")
                nc.scalar.dma_start(out=rhsNC[:], in_=rhsNC_d[b_])
                gtl10 = bat.tile([KG, NG], BF16, tag="gtl10")
                nc.gpsimd.dma_start(out=gtl10[:], in_=gtl10_d[b_])
                prhs10 = bat.tile([KG, NP], BF16, tag="prhs10")
                nc.gpsimd.dma_start(out=prhs10[:], in_=prhs10_d[b_])
                slf = bat.tile([128, NG], FP16, tag="slf")
                nc.sync.dma_start(out=slf[:], in_=slf_d[b_])
                nc.sync.dma_start(
                    out=pxyC[:, b_],
                    in_=ini[b_][:].rearrange("(m p) c -> p m c", m=NCH))
                nc.sync.dma_start(
                    out=pred2C[:, b_],
                    in_=pred2[b_][:].rearrange("(m p) c -> p m c", m=NCH))
                nc.sync.dma_start(
                    out=gtC[:, b_], in_=gt[b_][:].rearrange("(m p) c -> p m c", m=NCH))
                nc.sync.dma_start(
                    out=maskC[:, b_],
                    in_=kmask[b_][:].rearrange("(c p) -> p c", p=128))

                # ---------------- pred2gt (batch-wide tiles) ----------------
                eB = mrg.tile([128, NCH, NG], FP16, tag="eB")
                keyB = mrg.tile([128, NCH, NG], FP16, tag="keyB")
                psNCs = []
                for m in range(NCH):
                    sl = slice(128 * m, 128 * (m + 1))
                    psZ = pszp.tile([128, NG], F32, tag="psZ")
                    nc.tensor.matmul(psZ[:], lhsT=lhsZ[:, sl], rhs=rhsZ[:],
                                     start=True, stop=True)
                    psNC = psncp.tile([128, NG], F32, tag="psNC")
                    nc.tensor.matmul(psNC[:], lhsT=lhsNC[:, sl],
                                     rhs=rhsNC[:], start=True, stop=True)
                    psNCs.append(psNC)
                    # t* = clamp(round(z),0,9): fp16 +1024 rounding trick
                    c1 = drp.tile([128, NG], FP16, tag="c1")
                    nc.scalar.activation(out=c1[:], in_=psZ[:], func=AF.Relu)
                    c2 = drp.tile([128, NG], FP16, tag="c2")
                    nc.scalar.activation(out=c2[:], in_=c1[:], func=AF.Relu,
                                         bias=c9[:, 0:1], scale=-1.0)
                    yv = drp.tile([128, NG], FP16, tag="yv")
                    nc.scalar.activation(out=yv[:], in_=c2[:], func=AF.Copy,
                                         bias=1024.0)
                    # e = z - t = z + y - 1033
                    nc.vector.scalar_tensor_tensor(
                        out=eB[:, m], in0=yv[:], scalar=-1033.0,
                        in1=psZ[:], op0=ALU.add, op1=ALU.add)
                # batch-wide SBUF-only ops (one DVE instruction each)
                mB = mrg.tile([128, NCH, NG], FP16, tag="mB")
                nc.vector.tensor_tensor(
                    out=mB[:], in0=eB[:],
                    in1=slf[:].unsqueeze(1).to_broadcast([128, NCH, NG]),
                    op=ALU.mult)
                m2B = mrg.tile([128, NCH, NG], FP16, tag="m2B")
                nc.vector.tensor_tensor(out=m2B[:], in0=mB[:], in1=mB[:],
                                        op=ALU.mult)
                for m in range(NCH):
                    nc.vector.scalar_tensor_tensor(
                        out=keyB[:, m], in0=m2B[:, m], scalar=-1.0,
                        in1=psNCs[m][:], op0=ALU.mult, op1=ALU.add)
                mxB = small.tile([128, NCH], FP16, tag="mxB")
                nc.vector.tensor_reduce(out=mxB[:], in_=keyB[:], axis=AX.X,
                                        op=ALU.max)
                for m in range(NCH):
                    i8 = small.tile([128, 8], U32, tag="i8")
                    nc.vector.max_index(
                        out=i8[:],
                        in_max=mxB[:, m:m + 1].to_broadcast([128, 8]),
                        in_values=keyB[:, m])
                    nc.gpsimd.indirect_dma_start(
                        out=candC[:, b_, m].rearrange("p t c -> p (t c)"),
                        out_offset=None, in_=itab_d[:],
                        in_offset=IndirectOffsetOnAxis(ap=i8[:, 0:1],
                                                       axis=0),
                        element_offset=b_ * NG * T * 2)

                # ---------------- gt2pred ----------------
                key2B = mrg.tile([128, NCH, NP], FP16, tag="key2B")
                for c in range(NCH):
                    sl = slice(128 * c, 128 * (c + 1))
                    ps2 = psgp.tile([128, NP], F32, tag="ps2")
                    nc.tensor.matmul(ps2[:], lhsT=gtl10[:, sl], rhs=prhs10[:],
                                     start=True, stop=True)
                    nc.scalar.activation(out=key2B[:, c], in_=ps2[:],
                                         func=AF.Copy, bias=0.0)
                    if DEBUG and b_ == 0 and c == 0:
                        nc.sync.dma_start(out=dbg_pb[:], in_=key2B[:, c])
                gmxB = small.tile([128, NCH], FP16, tag="gmxB")
                nc.vector.tensor_reduce(out=gmxB[:], in_=key2B[:], axis=AX.X,
                                        op=ALU.max)
                for c in range(NCH):
                    ig8 = small.tile([128, 8], U32, tag="ig8")
                    nc.vector.max_index(
                        out=ig8[:],
                        in_max=gmxB[:, c:c + 1].to_broadcast([128, 8]),
                        in_values=key2B[:, c])
                    nc.gpsimd.indirect_dma_start(
                        out=npredC[:, b_, c, :], out_offset=None,
                        in_=ptab_d[:],
                        in_offset=IndirectOffsetOnAxis(ap=ig8[:, 0:1], axis=0),
                        element_offset=b_ * NP * 2)

            if DEBUG:
                nc.sync.dma_start(
                    out=dbg_candC[:],
                    in_=candC[:].rearrange("p b c t x -> p (b c t x)"))

            # ---------------- pred2gt refine + loss tail ----------------
            SH4 = [128, BLOC, NCH, T]
            dx = small.tile([128, BLOC, NCH, T], F32, tag="dx")
            dy = small.tile([128, BLOC, NCH, T], F32, tag="dy")
            nc.vector.tensor_tensor(
                out=dx[:], in0=candC[:, :, :, :, 0],
                in1=pxyC[:, :, :, 0:1].to_broadcast(SH4), op=ALU.subtract)
            nc.vector.tensor_tensor(
                out=dy[:], in0=candC[:, :, :, :, 1],
                in1=pxyC[:, :, :, 1:2].to_broadcast(SH4), op=ALU.subtract)
            sqx = small.tile([128, BLOC, NCH, T], F32, tag="sqx")
            sqy = small.tile([128, BLOC, NCH, T], F32, tag="sqy")
            dall = small.tile([128, BLOC, NCH, T], F32, tag="dall")
            nc.vector.tensor_tensor(out=sqx[:], in0=dx[:], in1=dx[:], op=ALU.mult)
            nc.vector.tensor_tensor(out=sqy[:], in0=dy[:], in1=dy[:], op=ALU.mult)
            nc.vector.tensor_tensor(out=dall[:], in0=sqx[:], in1=sqy[:], op=ALU.add)
            dmin = small.tile([128, BLOC, NCH], F32, tag="dmin")
            nc.vector.tensor_reduce(out=dmin[:], in_=dall[:], axis=AX.X, op=ALU.min)
            sel = small.tile([128, BLOC, NCH, T], F32, tag="sel")
            nc.vector.tensor_tensor(
                out=sel[:], in0=dall[:],
                in1=dmin[:].unsqueeze(3).to_broadcast(SH4), op=ALU.is_equal)
            selx = small.tile([128, BLOC, NCH, T], F32, tag="selx")
            sely = small.tile([128, BLOC, NCH, T], F32, tag="sely")
            nc.vector.tensor_tensor(out=selx[:], in0=sel[:],
                                    in1=candC[:, :, :, :, 0], op=ALU.mult)
            nc.vector.tensor_tensor(out=sely[:], in0=sel[:],
                                    in1=candC[:, :, :, :, 1], op=ALU.mult)
            # normalize by tie count to stay exact when two grid points tie
            cnt = small.tile([128, BLOC, NCH], F32, tag="cnt")
            nc.vector.tensor_reduce(out=cnt[:], in_=sel[:], axis=AX.X, op=ALU.add)
            rcnt = small.tile([128, BLOC, NCH], F32, tag="rcnt")
            nc.vector.reciprocal(out=rcnt[:], in_=cnt[:])
            nxs = small.tile([128, BLOC, NCH], F32, tag="nxs")
            nys = small.tile([128, BLOC, NCH], F32, tag="nys")
            nc.vector.tensor_reduce(out=nxs[:], in_=selx[:], axis=AX.X, op=ALU.add)
            nc.vector.tensor_reduce(out=nys[:], in_=sely[:], axis=AX.X, op=ALU.add)
            nx = small.tile([128, BLOC, NCH], F32, tag="nx")
            ny = small.tile([128, BLOC, NCH], F32, tag="ny")
            nc.vector.tensor_tensor(out=nx[:], in0=nxs[:], in1=rcnt[:], op=ALU.mult)
            nc.vector.tensor_tensor(out=ny[:], in0=nys[:], in1=rcnt[:], op=ALU.mult)
            df = small.tile([128, BLOC, NCH, 2], F32, tag="df")
            nc.vector.tensor_tensor(out=df[:, :, :, 0], in0=pred2C[:, :, :, 0],
                                    in1=nx[:], op=ALU.subtract)
            nc.vector.tensor_tensor(out=df[:, :, :, 1], in0=pred2C[:, :, :, 1],
                                    in1=ny[:], op=ALU.subtract)
            nc.vector.tensor_reduce(out=res[:, 0:BLOC], in_=df[:], axis=AX.XY,
                                    op=ALU.add, apply_absolute_value=True)

            # ---------------- gt2pred loss tail ----------------
            md = small.tile([128, BLOC, NCH, 2], F32, tag="md")
            nc.vector.tensor_tensor(out=md[:], in0=npredC[:], in1=gtC[:],
                                    op=ALU.subtract)
            sabs = small.tile([128, BLOC, NCH], F32, tag="sabs")
            nc.vector.tensor_reduce(out=sabs[:], in_=md[:], axis=AX.X,
                                    op=ALU.add, apply_absolute_value=True)
            smask = small.tile([128, BLOC, NCH], F32, tag="smask")
            nc.vector.tensor_tensor(out=smask[:], in0=sabs[:], in1=maskC[:],
                                    op=ALU.mult)
            nc.vector.tensor_reduce(out=res[:, 4:4 + BLOC], in_=smask[:],
                                    axis=AX.X, op=ALU.add)
            nc.vector.tensor_reduce(out=res[:, 8:8 + BLOC], in_=maskC[:],
                                    axis=AX.X, op=ALU.add)

            nc.sync.dma_start(out=out[:], in_=res[:])

    nc.compile()
    return nc


_NC_CACHE = None


def _get_nc():
    global _NC_CACHE
    if _NC_CACHE is None:
        _NC_CACHE = build_nc()
    return _NC_CACHE


def make_in_maps(ini_pred_poly, pred_polys_, gt_polys, keyPointsMask):
    in_maps = []
    for i in range(NCORES):
        s = slice(BLOC * i, BLOC * (i + 1))
        ini = np.ascontiguousarray(ini_pred_poly[s], dtype=np.float32)
        p2 = np.ascontiguousarray(pred_polys_[s], dtype=np.float32)
        gp = np.ascontiguousarray(gt_polys[s], dtype=np.float32)
        km = np.ascontiguousarray(keyPointsMask[s], dtype=np.float32)
        hp = host_prep(ini, p2, gp)
        im = {
            "ini_pred_poly": ini,
            "pred_polys_": p2,
            "gt_polys": gp,
            "keyPointsMask": km,
        }
        im.update(hp)
        in_maps.append(im)
    return in_maps


def combine_outputs(outs):
    """outs: list of [128, 12] per-core partial sums -> scalar loss (float32)."""
    acc = np.zeros(12, dtype=np.float64)
    for o in outs:
        acc += o.astype(np.float64).sum(axis=0)
    s_p2g = acc[0:4].sum()          # sum |pred_polys_ - nearest_gt|
    s_g2p = acc[4:8].sum()          # sum mask * |nearest_pred - gt|
    s_msk = 2.0 * acc[8:12].sum()   # sum of broadcast mask
    loss_pred2gt = s_p2g / (B * NP * 2)
    loss = (s_g2p / (s_msk + 1.0) + loss_pred2gt) / 2.0
    return np.float32(loss)


def kernel(ini_pred_poly, pred_polys_, gt_polys, keyPointsMask):
    nc = _get_nc()
    in_maps = make_in_maps(ini_pred_poly, pred_polys_, gt_polys, keyPointsMask)
    r = run_bass_kernel_spmd(nc, in_maps, list(range(NCORES)))
    return combine_outputs([r.results[i]["out"] for i in range(NCORES)])


if __name__ == "__main__":
    import reference

    inputs = {k: np.asarray(v) for k, v in reference.setup_inputs().items()}
    got = kernel(**inputs)
    print("kernel loss:", got)


# revision 39
# speedup vs baseline: 1.1163x; 1.1163x over previous
"""Trainium2 Bass kernel for nn_DMLoss_61942018343083 (Chamfer-style polygon
matching loss, retrieval_knn).

Sharding: data-parallel over batch B=32 across 8 NeuronCores (4 batches/core).
Each core computes partial sums into a [128, 12] output tile; the host combines
them into the scalar loss.

v3 design (exact-grid segment key in residual form):

pred2gt: for pred p and segment i (start v_i, dir s_i, 10 grid points at
  t/10), the exact grid-min key is
      key = -C^2 - ((z - t*) * L/10)^2,  t* = clamp(round(z), 0, 9)
  where C = perp line distance (linear form in p -> ONE K=17 bf16 monomial
  matmul gives -C^2 directly), z = 10(w.s)/len2 (K=8 bf16 matmul).
  t* via 3 scalar ACTs (Relu, 9-Relu, +1024 fp16 round trick: fp16 ulp at
  1024 is exactly 1, so the fp16 output write rounds c2 to an integer);
  the key via 4 DVE ops (residual STT reading psZ, x(L/10), square,
  -m^2+psNC STT). fp16 keys are safe because both -C^2 and the residual
  term are small near the argmax (no cancellation; far columns saturate to
  -inf which max/find handle fine). reduce(max) + find_index8 give the
  segment; a per-chunk indirect DMA (element_offset = per-batch bias into
  a per-core table) gathers the winning segment's 10 interp points; exact
  fp32 refine with tie-count normalization picks the true nearest.

gt2pred: K=10 bf16 matmul gives key2 = -|g-p|^2 directly (the -|g|^2 rows
  are folded into the matmul so values near the max are small ->
  fp16-safe, and no scalar bias ACT is needed); scalar Copy drains to
  fp16, reduce+find_index8 pick the nearest pred, indirect DMA gathers its
  pred_polys_ coords.

Empirical notes driving the design (from neuron-profile traces):
  - DVE TENSOR_REDUCE and FIND_INDEX8 run at 1 elem/cycle regardless of
    dtype here; TT/STT with all-SBUF 2-byte operands run 2x.
  - Indirect DMA costs ~994ns fixed + 0.34ns/descriptor on gpsimd; the 32
    per-chunk gathers (~36us busy) pipeline under the vector-bound main
    loop. Multi-offset (ap=[128,k>1]) indirect DMA mis-executes on HW
    (only the first offset column is honored) - do not batch gathers.
  - Slotted find_index8 (multiple chunk maxes in the 8 in_max slots over a
    concatenated in_values) is both slower and inaccurate on HW.
"""

import os
import sys

for _p in ("/opt/trn_rl_repo", "/root/.axon_site/_ro/trn_rl_repo"):
    if os.path.isdir(_p) and _p not in sys.path:
        sys.path.insert(0, _p)

import numpy as np
import ml_dtypes

bfloat16 = ml_dtypes.bfloat16

import concourse.bass as bass
import concourse.bacc as bacc
import concourse.mybir as mybir
from concourse.bass import IndirectOffsetOnAxis
from concourse.bass_utils import run_bass_kernel_spmd
from concourse.tile import TileContext

F32 = mybir.dt.float32
BF16 = mybir.dt.bfloat16
FP16 = mybir.dt.float16
U32 = mybir.dt.uint32
AF = mybir.ActivationFunctionType
ALU = mybir.AluOpType
AX = mybir.AxisListType

DEBUG = bool(int(os.environ.get("DMLOSS_DEBUG", "0")))

B, NP, NG, T = 32, 512, 512, 10
NCORES = 8
BLOC = B // NCORES          # 4 batches per core
NCH = NP // 128             # 4 chunks of 128 preds / 128 gts
CEN = np.float32(256.0)     # recentering shift
KZ = 8                      # z matmul contraction rows
KNC = 17                    # -C^2 monomial matmul rows
KG = 10                     # gt2pred contraction rows


def _split_hi_lo(x):
    x = np.asarray(x, dtype=np.float32)
    hi = x.astype(bfloat16)
    lo = (x - hi.astype(np.float32)).astype(bfloat16)
    return hi, lo


def host_prep(ini_pred_poly, pred_polys_, gt_polys):
    """Build all matmul operands / tables for one core's BLOC batches."""
    f = np.float32
    ini = np.asarray(ini_pred_poly, dtype=np.float32)   # [BLOC, NP, 2]
    p2 = np.asarray(pred_polys_, dtype=np.float32)      # [BLOC, NP, 2]
    gt = np.asarray(gt_polys, dtype=np.float32)         # [BLOC, NG, 2]
    v = np.roll(gt, 1, axis=1)
    s = (gt - v).astype(f)
    len2 = (s * s).sum(-1).astype(f)
    good = len2 > 1e-6
    L = np.sqrt(np.maximum(len2, 1e-12)).astype(f)
    inv = np.where(good, (f(10.0) / np.maximum(len2, f(1e-9))), f(0.0)).astype(f)

    pc = (ini - CEN).astype(f)
    vc = (v - CEN).astype(f)
    gc = (gt - CEN).astype(f)

    pxh, pxl = _split_hi_lo(pc[:, :, 0])
    pyh, pyl = _split_hi_lo(pc[:, :, 1])
    m1 = np.full_like(pxh, -1.0)
    one_p = np.ones_like(pxh)

    # ---- Z matmul: z = (p'.s - v'.s) * 10/len2 ----
    sx = (s[:, :, 0] * inv).astype(f)
    sy = (s[:, :, 1] * inv).astype(f)
    gs = ((vc * s).sum(-1) * inv).astype(f)
    sxh, sxl = _split_hi_lo(sx)
    syh, syl = _split_hi_lo(sy)
    gsh, gsl = _split_hi_lo(gs)
    lhsZ = np.stack([pxh, pxh, pxl, pyh, pyh, pyl, m1, m1],
                    axis=1).astype(bfloat16)                  # [BLOC, 8, NP]
    rhsZ = np.stack([sxh, sxl, sxh, syh, syl, syh, gsh, gsl],
                    axis=1).astype(bfloat16)                  # [BLOC, 8, NG]

    # ---- NC matmul: -C^2, C = a p'x + b p'y + c (perp line distance) ----
    with np.errstate(divide="ignore", invalid="ignore"):
        a = np.where(good, s[:, :, 1] / L, f(0.0)).astype(f)
        bco = np.where(good, -s[:, :, 0] / L, f(0.0)).astype(f)
    c0 = -(vc[:, :, 0] * a + vc[:, :, 1] * bco)
    # degenerate (zero-length) segments: kill their columns (covered by the
    # next segment's t=0 point)
    ck = np.where(good, c0, f(30000.0)).astype(f)

    X2 = (pc[:, :, 0] * pc[:, :, 0]).astype(f)
    XY = (pc[:, :, 0] * pc[:, :, 1]).astype(f)
    Y2 = (pc[:, :, 1] * pc[:, :, 1]).astype(f)
    A1 = (-(a * a)).astype(f)
    A2 = (-(2 * a * bco)).astype(f)
    A3 = (-(bco * bco)).astype(f)
    A4 = (-(2 * a * ck)).astype(f)
    A5 = (-(2 * bco * ck)).astype(f)
    A6 = (-(ck * ck)).astype(f)

    lhs_rows, rhs_rows = [], []
    for P, A in ((X2, A1), (XY, A2), (Y2, A3),
                 (pc[:, :, 0], A4), (pc[:, :, 1], A5)):
        Ph, Pl = _split_hi_lo(P)
        Ah, Al = _split_hi_lo(A)
        lhs_rows += [Ph, Ph, Pl]
        rhs_rows += [Ah, Al, Ah]
    A6h, A6l = _split_hi_lo(A6)
    lhs_rows += [one_p, one_p]
    rhs_rows += [A6h, A6l]
    lhsNC = np.stack(lhs_rows, axis=1).astype(bfloat16)       # [BLOC, 17, NP]
    rhsNC = np.stack(rhs_rows, axis=1).astype(bfloat16)       # [BLOC, 17, NG]

    # ---- gt2pred: key2 = 2g'.p' - |p'|^2 - |g'|^2 = -d^2 ----
    g2xh, g2xl = _split_hi_lo(f(2.0) * gc[:, :, 0])
    g2yh, g2yl = _split_hi_lo(f(2.0) * gc[:, :, 1])
    m1g = np.full_like(g2xh, -1.0)
    ngh, ngl = _split_hi_lo(-(gc * gc).sum(-1))
    gtl10 = np.stack([g2xh, g2xh, g2xl, g2yh, g2yh, g2yl, m1g, m1g, ngh, ngl],
                     axis=1).astype(bfloat16)                 # [BLOC, 10, NG]
    pp = (pc * pc).sum(-1).astype(f)
    pph, ppl = _split_hi_lo(pp)
    prhs10 = np.stack([pxh, pxl, pxh, pyh, pyl, pyh, pph, ppl, one_p, one_p],
                      axis=1).astype(bfloat16)                # [BLOC, 10, NP]

    # ---- slf: L/10 per column, replicated across partitions, fp16 ----
    sl = (L / f(10.0)).astype(np.float16)                     # [BLOC, NG]
    slf = np.broadcast_to(sl[:, None, :], (BLOC, 128, NG)).copy()

    # ---- interp table, bit-exact ref math: [BLOC*NG, T*2] f32 ----
    a_t = (np.arange(T, dtype=f) / f(T)).astype(f)
    itab = np.empty((BLOC, NG, T, 2), dtype=f)
    for t in range(T):
        itab[:, :, t, :] = (gt * a_t[t]).astype(f) + (v * (f(1.0) - a_t[t])).astype(f)
    itabAll = itab.reshape(BLOC * NG, T * 2)

    # ---- pred table for gt2pred gather: [BLOC*NP, 2] f32 ----
    ptabAll = p2.reshape(BLOC * NP, 2).astype(f)

    return dict(lhsZ=lhsZ, rhsZ=rhsZ, lhsNC=lhsNC, rhsNC=rhsNC,
                gtl10=gtl10, prhs10=prhs10, slf=slf,
                itabAll=itabAll, ptabAll=ptabAll)


def build_nc():
    nc = bacc.Bacc()

    ini = nc.dram_tensor("ini_pred_poly", [BLOC, NP, 2], F32, kind="ExternalInput")
    pred2 = nc.dram_tensor("pred_polys_", [BLOC, NP, 2], F32, kind="ExternalInput")
    gt = nc.dram_tensor("gt_polys", [BLOC, NG, 2], F32, kind="ExternalInput")
    kmask = nc.dram_tensor("keyPointsMask", [BLOC, NG], F32, kind="ExternalInput")
    lhsZ_d = nc.dram_tensor("lhsZ", [BLOC, KZ, NP], BF16, kind="ExternalInput")
    rhsZ_d = nc.dram_tensor("rhsZ", [BLOC, KZ, NG], BF16, kind="ExternalInput")
    lhsNC_d = nc.dram_tensor("lhsNC", [BLOC, KNC, NP], BF16, kind="ExternalInput")
    rhsNC_d = nc.dram_tensor("rhsNC", [BLOC, KNC, NG], BF16, kind="ExternalInput")
    gtl10_d = nc.dram_tensor("gtl10", [BLOC, KG, NG], BF16, kind="ExternalInput")
    prhs10_d = nc.dram_tensor("prhs10", [BLOC, KG, NP], BF16, kind="ExternalInput")
    slf_d = nc.dram_tensor("slf", [BLOC, 128, NG], FP16, kind="ExternalInput")
    itab_d = nc.dram_tensor("itabAll", [BLOC * NG, T * 2], F32, kind="ExternalInput")
    ptab_d = nc.dram_tensor("ptabAll", [BLOC * NP, 2], F32, kind="ExternalInput")
    out = nc.dram_tensor("out", [128, 12], F32, kind="ExternalOutput")
    if DEBUG:
        dbg_key = nc.dram_tensor("dbg_key", [128, NG], FP16, kind="ExternalOutput")
        dbg_pb = nc.dram_tensor("dbg_pb", [128, NP], FP16, kind="ExternalOutput")
        dbg_candC = nc.dram_tensor("dbg_candC", [128, BLOC * NCH * T * 2], F32, kind="ExternalOutput")

    NSL = BLOC * NCH  # 16 (batch, chunk) slots

    with TileContext(nc) as tc:
        with (
            tc.tile_pool(name="const", bufs=1) as cpool,
            tc.tile_pool(name="bat", bufs=2) as bat,
            tc.tile_pool(name="drp", bufs=5) as drp,
            tc.tile_pool(name="mrg", bufs=4) as mrg,
            tc.tile_pool(name="small", bufs=5) as small,
            tc.tile_pool(name="psZ", bufs=2, space="PSUM") as pszp,
            tc.tile_pool(name="psNC", bufs=2, space="PSUM") as psncp,
            tc.tile_pool(name="psG", bufs=2, space="PSUM") as psgp,
        ):
            res = cpool.tile([128, 12], F32)
            nc.vector.memset(res[:], 0.0)
            c9 = cpool.tile([128, 1], F32)
            nc.vector.memset(c9[:], 9.0)
            candC = cpool.tile([128, BLOC, NCH, T, 2], F32)
            npredC = cpool.tile([128, BLOC, NCH, 2], F32)
            pxyC = cpool.tile([128, BLOC, NCH, 2], F32)
            pred2C = cpool.tile([128, BLOC, NCH, 2], F32)
            gtC = cpool.tile([128, BLOC, NCH, 2], F32)
            maskC = cpool.tile([128, BLOC, NCH], F32)


            for b_ in range(BLOC):
                lhsZ = bat.tile([KZ, NP], BF16, tag="lhsZ")
                nc.sync.dma_start(out=lhsZ[:], in_=lhsZ_d[b_])
                rhsZ = bat.tile([KZ, NG], BF16, tag="rhsZ")
                nc.sync.dma_start(out=rhsZ[:], in_=rhsZ_d[b_])
                lhsNC = bat.tile([KNC, NP], BF16, tag="lhsNC")
                nc.scalar.dma_start(out=lhsNC[:], in_=lhsNC_d[b_])
                rhsNC = bat.tile([KNC, NG], BF16, tag="rhsNC")
                nc.scalar.dma_start(out=rhsNC[:], in_=rhsNC_d[b_])
                gtl10 = bat.tile([KG, NG], BF16, tag="gtl10")
                nc.gpsimd.dma_start(out=gtl10[:], in_=gtl10_d[b_])
                prhs10 = bat.tile([KG, NP], BF16, tag="prhs10")
                nc.gpsimd.dma_start(out=prhs10[:], in_=prhs10_d[b_])
                slf = bat.tile([128, NG], FP16, tag="slf")
                nc.sync.dma_start(out=slf[:], in_=slf_d[b_])
                nc.sync.dma_start(
                    out=pxyC[:, b_],
                    in_=ini[b_][:].rearrange("(m p) c -> p m c", m=NCH))
                nc.sync.dma_start(
                    out=pred2C[:, b_],
                    in_=pred2[b_][:].rearrange("(m p) c -> p m c", m=NCH))
                nc.sync.dma_start(
                    out=gtC[:, b_], in_=gt[b_][:].rearrange("(m p) c -> p m c", m=NCH))
                nc.sync.dma_start(
                    out=maskC[:, b_],
                    in_=kmask[b_][:].rearrange("(c p) -> p c", p=128))

                # ---------------- pred2gt (chunk pairs) ----------------
                for mp in range(NCH // 2):
                    eP = mrg.tile([128, 2, NG], FP16, tag="eP")
                    keyP = mrg.tile([128, 2, NG], FP16, tag="keyP")
                    psNCs = []
                    for mi in range(2):
                        m = 2 * mp + mi
                        sl = slice(128 * m, 128 * (m + 1))
                        psZ = pszp.tile([128, NG], F32, tag="psZ")
                        nc.tensor.matmul(psZ[:], lhsT=lhsZ[:, sl], rhs=rhsZ[:],
                                         start=True, stop=True)
                        psNC = psncp.tile([128, NG], F32, tag="psNC")
                        nc.tensor.matmul(psNC[:], lhsT=lhsNC[:, sl],
                                         rhs=rhsNC[:], start=True, stop=True)
                        psNCs.append(psNC)
                        # t* = clamp(round(z),0,9): fp16 +1024 rounding trick
                        c1 = drp.tile([128, NG], FP16, tag="c1")
                        nc.scalar.activation(out=c1[:], in_=psZ[:], func=AF.Relu)
                        c2 = drp.tile([128, NG], FP16, tag="c2")
                        nc.scalar.activation(out=c2[:], in_=c1[:], func=AF.Relu,
                                             bias=c9[:, 0:1], scale=-1.0)
                        yv = drp.tile([128, NG], FP16, tag="yv")
                        nc.scalar.activation(out=yv[:], in_=c2[:], func=AF.Copy,
                                             bias=1024.0)
                        # e = z - t = z + y - 1033
                        nc.vector.scalar_tensor_tensor(
                            out=eP[:, mi], in0=yv[:], scalar=-1033.0,
                            in1=psZ[:], op0=ALU.add, op1=ALU.add)
                    # pairwise SBUF-only ops run double-width
                    mP = mrg.tile([128, 2, NG], FP16, tag="mP")
                    nc.vector.tensor_tensor(
                        out=mP[:], in0=eP[:],
                        in1=slf[:].unsqueeze(1).to_broadcast([128, 2, NG]),
                        op=ALU.mult)
                    m2P = mrg.tile([128, 2, NG], FP16, tag="m2P")
                    nc.vector.tensor_tensor(out=m2P[:], in0=mP[:], in1=mP[:],
                                            op=ALU.mult)
                    for mi in range(2):
                        nc.vector.scalar_tensor_tensor(
                            out=keyP[:, mi], in0=m2P[:, mi], scalar=-1.0,
                            in1=psNCs[mi][:], op0=ALU.mult, op1=ALU.add)
                    mxP = small.tile([128, 2], FP16, tag="mxP")
                    nc.vector.tensor_reduce(out=mxP[:], in_=keyP[:], axis=AX.X,
                                            op=ALU.max)
                    for mi in range(2):
                        m = 2 * mp + mi
                        i8 = small.tile([128, 8], U32, tag="i8")
                        nc.vector.max_index(
                            out=i8[:],
                            in_max=mxP[:, mi:mi + 1].to_broadcast([128, 8]),
                            in_values=keyP[:, mi])
                        nc.gpsimd.indirect_dma_start(
                            out=candC[:, b_, m].rearrange("p t c -> p (t c)"),
                            out_offset=None, in_=itab_d[:],
                            in_offset=IndirectOffsetOnAxis(ap=i8[:, 0:1],
                                                           axis=0),
                            element_offset=b_ * NG * T * 2)

                # ---------------- gt2pred ----------------
                for c in range(NCH):
                    sl = slice(128 * c, 128 * (c + 1))
                    ps2 = psgp.tile([128, NP], F32, tag="ps2")
                    nc.tensor.matmul(ps2[:], lhsT=gtl10[:, sl], rhs=prhs10[:],
                                     start=True, stop=True)
                    key2 = mrg.tile([128, NP], FP16, tag="key2")
                    nc.scalar.activation(out=key2[:], in_=ps2[:], func=AF.Copy,
                                         bias=0.0)
                    gmx = small.tile([128, 1], FP16, tag="gmx")
                    nc.vector.tensor_reduce(out=gmx[:], in_=key2[:], axis=AX.X,
                                            op=ALU.max)
                    ig8 = small.tile([128, 8], U32, tag="ig8")
                    nc.vector.max_index(out=ig8[:],
                                        in_max=gmx[:].to_broadcast([128, 8]),
                                        in_values=key2[:])
                    nc.gpsimd.indirect_dma_start(
                        out=npredC[:, b_, c, :], out_offset=None,
                        in_=ptab_d[:],
                        in_offset=IndirectOffsetOnAxis(ap=ig8[:, 0:1], axis=0),
                        element_offset=b_ * NP * 2)
                    if DEBUG and b_ == 0 and c == 0:
                        nc.sync.dma_start(out=dbg_pb[:], in_=key2[:])

            if DEBUG:
                nc.sync.dma_start(
                    out=dbg_candC[:],
                    in_=candC[:].rearrange("p b c t x -> p (b c t x)"))

            # ---------------- pred2gt refine + loss tail ----------------
            SH4 = [128, BLOC, NCH, T]
            dx = small.tile([128, BLOC, NCH, T], F32, tag="dx")
            dy = small.tile([128, BLOC, NCH, T], F32, tag="dy")
            nc.vector.tensor_tensor(
                out=dx[:], in0=candC[:, :, :, :, 0],
                in1=pxyC[:, :, :, 0:1].to_broadcast(SH4), op=ALU.subtract)
            nc.vector.tensor_tensor(
                out=dy[:], in0=candC[:, :, :, :, 1],
                in1=pxyC[:, :, :, 1:2].to_broadcast(SH4), op=ALU.subtract)
            sqx = small.tile([128, BLOC, NCH, T], F32, tag="sqx")
            sqy = small.tile([128, BLOC, NCH, T], F32, tag="sqy")
            dall = small.tile([128, BLOC, NCH, T], F32, tag="dall")
            nc.vector.tensor_tensor(out=sqx[:], in0=dx[:], in1=dx[:], op=ALU.mult)
            nc.vector.tensor_tensor(out=sqy[:], in0=dy[:], in1=dy[:], op=ALU.mult)
            nc.vector.tensor_tensor(out=dall[:], in0=sqx[:], in1=sqy[:], op=ALU.add)
            dmin = small.tile([128, BLOC, NCH], F32, tag="dmin")
            nc.vector.tensor_reduce(out=dmin[:], in_=dall[:], axis=AX.X, op=ALU.min)
            sel = small.tile([128, BLOC, NCH, T], F32, tag="sel")
            nc.vector.tensor_tensor(
                out=sel[:], in0=dall[:],
                in1=dmin[:].unsqueeze(3).to_broadcast(SH4), op=ALU.is_equal)
            selx = small.tile([128, BLOC, NCH, T], F32, tag="selx")
            sely = small.tile([128, BLOC, NCH, T], F32, tag="sely")
            nc.vector.tensor_tensor(out=selx[:], in0=sel[:],
                                    in1=candC[:, :, :, :, 0], op=ALU.mult)
            nc.vector.tensor_tensor(out=sely[:], in0=sel[:],
                                    in1=candC[:, :, :, :, 1], op=ALU.mult)
            # normalize by tie count to stay exact when two grid points tie
            cnt = small.tile([128, BLOC, NCH], F32, tag="cnt")
            nc.vector.tensor_reduce(out=cnt[:], in_=sel[:], axis=AX.X, op=ALU.add)
            rcnt = small.tile([128, BLOC, NCH], F32, tag="rcnt")
            nc.vector.reciprocal(out=rcnt[:], in_=cnt[:])
            nxs = small.tile([128, BLOC, NCH], F32, tag="nxs")
            nys = small.tile([128, BLOC, NCH], F32, tag="nys")
            nc.vector.tensor_reduce(out=nxs[:], in_=selx[:], axis=AX.X, op=ALU.add)
            nc.vector.tensor_reduce(out=nys[:], in_=sely[:], axis=AX.X, op=ALU.add)
            nx = small.tile([128, BLOC, NCH], F32, tag="nx")
            ny = small.tile([128, BLOC, NCH], F32, tag="ny")
            nc.vector.tensor_tensor(out=nx[:], in0=nxs[:], in1=rcnt[:], op=ALU.mult)
            nc.vector.tensor_tensor(out=ny[:], in0=nys[:], in1=rcnt[:], op=ALU.mult)
            df = small.tile([128, BLOC, NCH, 2], F32, tag="df")
            nc.vector.tensor_tensor(out=df[:, :, :, 0], in0=pred2C[:, :, :, 0],
                                    in1=nx[:], op=ALU.subtract)
            nc.vector.tensor_tensor(out=df[:, :, :, 1], in0=pred2C[:, :, :, 1],
                                    in1=ny[:], op=ALU.subtract)
            nc.vector.tensor_reduce(out=res[:, 0:BLOC], in_=df[:], axis=AX.XY,
                                    op=ALU.add, apply_absolute_value=True)

            # ---------------- gt2pred loss tail ----------------
            md = small.tile([128, BLOC, NCH, 2], F32, tag="md")
            nc.vector.tensor_tensor(out=md[:], in0=npredC[:], in1=gtC[:],
                                    op=ALU.subtract)
            sabs = small.tile([128, BLOC, NCH], F32, tag="sabs")
            nc.vector.tensor_reduce(out=sabs[:], in_=md[:], axis=AX.X,
                                    op=ALU.add, apply_absolute_value=True)
            smask = small.tile([128, BLOC, NCH], F32, tag="smask")
            nc.vector.tensor_tensor(out=smask[:], in0=sabs[:], in1=maskC[:],
                                    op=ALU.mult)
            nc.vector.tensor_reduce(out=res[:, 4:4 + BLOC], in_=smask[:],
                                    axis=AX.X, op=ALU.add)
            nc.vector.tensor_reduce(out=res[:, 8:8 + BLOC], in_=maskC[:],
                                    axis=AX.X, op=ALU.add)

            nc.sync.dma_start(out=out[:], in_=res[:])

    nc.compile()
    return nc


_NC_CACHE = None


def _get_nc():
    global _NC_CACHE
    if _NC_CACHE is None:
        _NC_CACHE = build_nc()
    return _NC_CACHE


def make_in_maps(ini_pred_poly, pred_polys_, gt_polys, keyPointsMask):
    in_maps = []
    for i in range(NCORES):
        s = slice(BLOC * i, BLOC * (i + 1))
        ini = np.ascontiguousarray(ini_pred_poly[s], dtype=np.float32)
        p2 = np.ascontiguousarray(pred_polys_[s], dtype=np.float32)
        gp = np.ascontiguousarray(gt_polys[s], dtype=np.float32)
        km = np.ascontiguousarray(keyPointsMask[s], dtype=np.float32)
        hp = host_prep(ini, p2, gp)
        im = {
            "ini_pred_poly": ini,
            "pred_polys_": p2,
            "gt_polys": gp,
            "keyPointsMask": km,
        }
        im.update(hp)
        in_maps.append(im)
    return in_maps


def combine_outputs(outs):
    """outs: list of [128, 12] per-core partial sums -> scalar loss (float32)."""
    acc = np.zeros(12, dtype=np.float64)
    for o in outs:
        acc += o.astype(np.float64).sum(axis=0)
    s_p2g = acc[0:4].sum()          # sum |pred_polys_ - nearest_gt|
    s_g2p = acc[4:8].sum()          # sum mask * |nearest_pred - gt|
    s_msk = 2.0 * acc[8:12].sum()   # sum of broadcast mask
    loss_pred2gt = s_p2g / (B * NP * 2)
    loss = (s_g2p / (s_msk + 1.0) + loss_pred2gt) / 2.0
    return np.float32(loss)


def kernel(ini_pred_poly, pred_polys_, gt_polys, keyPointsMask):
    nc = _get_nc()
    in_maps = make_in_maps(ini_pred_poly, pred_polys_, gt_polys, keyPointsMask)
    r = run_bass_kernel_spmd(nc, in_maps, list(range(NCORES)))
    return combine_outputs([r.results[i]["out"] for i in range(NCORES)])


if __name__ == "__main__":
    import reference

    inputs = {k: np.asarray(v) for k, v in reference.setup_inputs().items()}
    got = kernel(**inputs)
    print("kernel loss:", got)


# revision 41
# speedup vs baseline: 1.1264x; 1.0090x over previous
"""Trainium2 Bass kernel for nn_DMLoss_61942018343083 (Chamfer-style polygon
matching loss, retrieval_knn).

Sharding: data-parallel over batch B=32 across 8 NeuronCores (4 batches/core).
Each core computes partial sums into a [128, 12] output tile; the host combines
them into the scalar loss.

v3 design (exact-grid segment key in residual form):

pred2gt: for pred p and segment i (start v_i, dir s_i, 10 grid points at
  t/10), the exact grid-min key is
      key = -C^2 - ((z - t*) * L/10)^2,  t* = clamp(round(z), 0, 9)
  where C = perp line distance (linear form in p -> ONE K=17 bf16 monomial
  matmul gives -C^2 directly), z = 10(w.s)/len2 (K=8 bf16 matmul).
  t* via 3 scalar ACTs (Relu, 9-Relu, +1024 fp16 round trick: fp16 ulp at
  1024 is exactly 1, so the fp16 output write rounds c2 to an integer);
  the key via 4 DVE ops (residual STT reading psZ, x(L/10), square,
  -m^2+psNC STT). fp16 keys are safe because both -C^2 and the residual
  term are small near the argmax (no cancellation; far columns saturate to
  -inf which max/find handle fine). reduce(max) + find_index8 give the
  segment; a per-chunk indirect DMA (element_offset = per-batch bias into
  a per-core table) gathers the winning segment's 10 interp points; exact
  fp32 refine with tie-count normalization picks the true nearest.

gt2pred: K=10 bf16 matmul gives key2 = -|g-p|^2 directly (the -|g|^2 rows
  are folded into the matmul so values near the max are small ->
  fp16-safe, and no scalar bias ACT is needed); scalar Copy drains to
  fp16, reduce+find_index8 pick the nearest pred, indirect DMA gathers its
  pred_polys_ coords.

Empirical notes driving the design (from neuron-profile traces):
  - DVE TENSOR_REDUCE and FIND_INDEX8 run at 1 elem/cycle regardless of
    dtype here; TT/STT with all-SBUF 2-byte operands run 2x.
  - Indirect DMA costs ~994ns fixed + 0.34ns/descriptor on gpsimd; the 32
    per-chunk gathers (~36us busy) pipeline under the vector-bound main
    loop. Multi-offset (ap=[128,k>1]) indirect DMA mis-executes on HW
    (only the first offset column is honored) - do not batch gathers.
  - Slotted find_index8 (multiple chunk maxes in the 8 in_max slots over a
    concatenated in_values) is both slower and inaccurate on HW.
"""

import os
import sys

for _p in ("/opt/trn_rl_repo", "/root/.axon_site/_ro/trn_rl_repo"):
    if os.path.isdir(_p) and _p not in sys.path:
        sys.path.insert(0, _p)

import numpy as np
import ml_dtypes

bfloat16 = ml_dtypes.bfloat16

import concourse.bass as bass
import concourse.bacc as bacc
import concourse.mybir as mybir
from concourse.bass import IndirectOffsetOnAxis
from concourse.bass_utils import run_bass_kernel_spmd
from concourse.tile import TileContext

F32 = mybir.dt.float32
BF16 = mybir.dt.bfloat16
FP16 = mybir.dt.float16
U32 = mybir.dt.uint32
AF = mybir.ActivationFunctionType
ALU = mybir.AluOpType
AX = mybir.AxisListType

DEBUG = bool(int(os.environ.get("DMLOSS_DEBUG", "0")))

B, NP, NG, T = 32, 512, 512, 10
NCORES = 8
BLOC = B // NCORES          # 4 batches per core
NCH = NP // 128             # 4 chunks of 128 preds / 128 gts
CEN = np.float32(256.0)     # recentering shift
KZ = 8                      # z matmul contraction rows
KNC = 17                    # -C^2 monomial matmul rows
KG = 10                     # gt2pred contraction rows


def _split_hi_lo(x):
    x = np.asarray(x, dtype=np.float32)
    hi = x.astype(bfloat16)
    lo = (x - hi.astype(np.float32)).astype(bfloat16)
    return hi, lo


def host_prep(ini_pred_poly, pred_polys_, gt_polys):
    """Build all matmul operands / tables for one core's BLOC batches."""
    f = np.float32
    ini = np.asarray(ini_pred_poly, dtype=np.float32)   # [BLOC, NP, 2]
    p2 = np.asarray(pred_polys_, dtype=np.float32)      # [BLOC, NP, 2]
    gt = np.asarray(gt_polys, dtype=np.float32)         # [BLOC, NG, 2]
    v = np.roll(gt, 1, axis=1)
    s = (gt - v).astype(f)
    len2 = (s * s).sum(-1).astype(f)
    good = len2 > 1e-6
    L = np.sqrt(np.maximum(len2, 1e-12)).astype(f)
    inv = np.where(good, (f(10.0) / np.maximum(len2, f(1e-9))), f(0.0)).astype(f)

    pc = (ini - CEN).astype(f)
    vc = (v - CEN).astype(f)
    gc = (gt - CEN).astype(f)

    pxh, pxl = _split_hi_lo(pc[:, :, 0])
    pyh, pyl = _split_hi_lo(pc[:, :, 1])
    m1 = np.full_like(pxh, -1.0)
    one_p = np.ones_like(pxh)

    # ---- Z matmul: z = (p'.s - v'.s) * 10/len2 ----
    sx = (s[:, :, 0] * inv).astype(f)
    sy = (s[:, :, 1] * inv).astype(f)
    gs = ((vc * s).sum(-1) * inv).astype(f)
    sxh, sxl = _split_hi_lo(sx)
    syh, syl = _split_hi_lo(sy)
    gsh, gsl = _split_hi_lo(gs)
    lhsZ = np.stack([pxh, pxh, pxl, pyh, pyh, pyl, m1, m1],
                    axis=1).astype(bfloat16)                  # [BLOC, 8, NP]
    rhsZ = np.stack([sxh, sxl, sxh, syh, syl, syh, gsh, gsl],
                    axis=1).astype(bfloat16)                  # [BLOC, 8, NG]

    # ---- NC matmul: -C^2, C = a p'x + b p'y + c (perp line distance) ----
    with np.errstate(divide="ignore", invalid="ignore"):
        a = np.where(good, s[:, :, 1] / L, f(0.0)).astype(f)
        bco = np.where(good, -s[:, :, 0] / L, f(0.0)).astype(f)
    c0 = -(vc[:, :, 0] * a + vc[:, :, 1] * bco)
    # degenerate (zero-length) segments: kill their columns (covered by the
    # next segment's t=0 point)
    ck = np.where(good, c0, f(30000.0)).astype(f)

    X2 = (pc[:, :, 0] * pc[:, :, 0]).astype(f)
    XY = (pc[:, :, 0] * pc[:, :, 1]).astype(f)
    Y2 = (pc[:, :, 1] * pc[:, :, 1]).astype(f)
    A1 = (-(a * a)).astype(f)
    A2 = (-(2 * a * bco)).astype(f)
    A3 = (-(bco * bco)).astype(f)
    A4 = (-(2 * a * ck)).astype(f)
    A5 = (-(2 * bco * ck)).astype(f)
    A6 = (-(ck * ck)).astype(f)

    lhs_rows, rhs_rows = [], []
    for P, A in ((X2, A1), (XY, A2), (Y2, A3),
                 (pc[:, :, 0], A4), (pc[:, :, 1], A5)):
        Ph, Pl = _split_hi_lo(P)
        Ah, Al = _split_hi_lo(A)
        lhs_rows += [Ph, Ph, Pl]
        rhs_rows += [Ah, Al, Ah]
    A6h, A6l = _split_hi_lo(A6)
    lhs_rows += [one_p, one_p]
    rhs_rows += [A6h, A6l]
    lhsNC = np.stack(lhs_rows, axis=1).astype(bfloat16)       # [BLOC, 17, NP]
    rhsNC = np.stack(rhs_rows, axis=1).astype(bfloat16)       # [BLOC, 17, NG]

    # ---- gt2pred: key2 = 2g'.p' - |p'|^2 - |g'|^2 = -d^2 ----
    g2xh, g2xl = _split_hi_lo(f(2.0) * gc[:, :, 0])
    g2yh, g2yl = _split_hi_lo(f(2.0) * gc[:, :, 1])
    m1g = np.full_like(g2xh, -1.0)
    ngh, ngl = _split_hi_lo(-(gc * gc).sum(-1))
    gtl10 = np.stack([g2xh, g2xh, g2xl, g2yh, g2yh, g2yl, m1g, m1g, ngh, ngl],
                     axis=1).astype(bfloat16)                 # [BLOC, 10, NG]
    pp = (pc * pc).sum(-1).astype(f)
    pph, ppl = _split_hi_lo(pp)
    prhs10 = np.stack([pxh, pxl, pxh, pyh, pyl, pyh, pph, ppl, one_p, one_p],
                      axis=1).astype(bfloat16)                # [BLOC, 10, NP]

    # ---- slf: L/10 per column, replicated across partitions, fp16 ----
    sl = (L / f(10.0)).astype(np.float16)                     # [BLOC, NG]
    slf = np.broadcast_to(sl[:, None, :], (BLOC, 128, NG)).copy()

    # ---- interp table, bit-exact ref math: [BLOC*NG, T*2] f32 ----
    a_t = (np.arange(T, dtype=f) / f(T)).astype(f)
    itab = np.empty((BLOC, NG, T, 2), dtype=f)
    for t in range(T):
        itab[:, :, t, :] = (gt * a_t[t]).astype(f) + (v * (f(1.0) - a_t[t])).astype(f)
    itabAll = itab.reshape(BLOC * NG, T * 2)

    # ---- pred table for gt2pred gather: [BLOC*NP, 2] f32 ----
    ptabAll = p2.reshape(BLOC * NP, 2).astype(f)

    return dict(lhsZ=lhsZ, rhsZ=rhsZ, lhsNC=lhsNC, rhsNC=rhsNC,
                gtl10=gtl10, prhs10=prhs10, slf=slf,
                itabAll=itabAll, ptabAll=ptabAll)


def build_nc():
    nc = bacc.Bacc()

    ini = nc.dram_tensor("ini_pred_poly", [BLOC, NP, 2], F32, kind="ExternalInput")
    pred2 = nc.dram_tensor("pred_polys_", [BLOC, NP, 2], F32, kind="ExternalInput")
    gt = nc.dram_tensor("gt_polys", [BLOC, NG, 2], F32, kind="ExternalInput")
    kmask = nc.dram_tensor("keyPointsMask", [BLOC, NG], F32, kind="ExternalInput")
    lhsZ_d = nc.dram_tensor("lhsZ", [BLOC, KZ, NP], BF16, kind="ExternalInput")
    rhsZ_d = nc.dram_tensor("rhsZ", [BLOC, KZ, NG], BF16, kind="ExternalInput")
    lhsNC_d = nc.dram_tensor("lhsNC", [BLOC, KNC, NP], BF16, kind="ExternalInput")
    rhsNC_d = nc.dram_tensor("rhsNC", [BLOC, KNC, NG], BF16, kind="ExternalInput")
    gtl10_d = nc.dram_tensor("gtl10", [BLOC, KG, NG], BF16, kind="ExternalInput")
    prhs10_d = nc.dram_tensor("prhs10", [BLOC, KG, NP], BF16, kind="ExternalInput")
    slf_d = nc.dram_tensor("slf", [BLOC, 128, NG], FP16, kind="ExternalInput")
    itab_d = nc.dram_tensor("itabAll", [BLOC * NG, T * 2], F32, kind="ExternalInput")
    ptab_d = nc.dram_tensor("ptabAll", [BLOC * NP, 2], F32, kind="ExternalInput")
    out = nc.dram_tensor("out", [128, 12], F32, kind="ExternalOutput")
    if DEBUG:
        dbg_key = nc.dram_tensor("dbg_key", [128, NG], FP16, kind="ExternalOutput")
        dbg_pb = nc.dram_tensor("dbg_pb", [128, NP], FP16, kind="ExternalOutput")
        dbg_candC = nc.dram_tensor("dbg_candC", [128, BLOC * NCH * T * 2], F32, kind="ExternalOutput")

    NSL = BLOC * NCH  # 16 (batch, chunk) slots

    with TileContext(nc) as tc:
        with (
            tc.tile_pool(name="const", bufs=1) as cpool,
            tc.tile_pool(name="bat", bufs=2) as bat,
            tc.tile_pool(name="drp", bufs=5) as drp,
            tc.tile_pool(name="mrg", bufs=4) as mrg,
            tc.tile_pool(name="small", bufs=5) as small,
            tc.tile_pool(name="psZ", bufs=2, space="PSUM") as pszp,
            tc.tile_pool(name="psNC", bufs=2, space="PSUM") as psncp,
            tc.tile_pool(name="psG", bufs=2, space="PSUM") as psgp,
        ):
            res = cpool.tile([128, 12], F32)
            nc.vector.memset(res[:], 0.0)
            c9 = cpool.tile([128, 1], F32)
            nc.vector.memset(c9[:], 9.0)
            candC = cpool.tile([128, BLOC, NCH, 2 * T, 2], F32)
            npredC = cpool.tile([128, BLOC, NCH, 2], F32)
            pxyC = cpool.tile([128, BLOC, NCH, 2], F32)
            pred2C = cpool.tile([128, BLOC, NCH, 2], F32)
            gtC = cpool.tile([128, BLOC, NCH, 2], F32)
            maskC = cpool.tile([128, BLOC, NCH], F32)


            for b_ in range(BLOC):
                lhsZ = bat.tile([KZ, NP], BF16, tag="lhsZ")
                nc.sync.dma_start(out=lhsZ[:], in_=lhsZ_d[b_])
                rhsZ = bat.tile([KZ, NG], BF16, tag="rhsZ")
                nc.sync.dma_start(out=rhsZ[:], in_=rhsZ_d[b_])
                lhsNC = bat.tile([KNC, NP], BF16, tag="lhsNC")
                nc.scalar.dma_start(out=lhsNC[:], in_=lhsNC_d[b_])
                rhsNC = bat.tile([KNC, NG], BF16, tag="rhsNC")
                nc.scalar.dma_start(out=rhsNC[:], in_=rhsNC_d[b_])
                gtl10 = bat.tile([KG, NG], BF16, tag="gtl10")
                nc.gpsimd.dma_start(out=gtl10[:], in_=gtl10_d[b_])
                prhs10 = bat.tile([KG, NP], BF16, tag="prhs10")
                nc.gpsimd.dma_start(out=prhs10[:], in_=prhs10_d[b_])
                slf = bat.tile([128, NG], FP16, tag="slf")
                nc.sync.dma_start(out=slf[:], in_=slf_d[b_])
                nc.sync.dma_start(
                    out=pxyC[:, b_],
                    in_=ini[b_][:].rearrange("(m p) c -> p m c", m=NCH))
                nc.sync.dma_start(
                    out=pred2C[:, b_],
                    in_=pred2[b_][:].rearrange("(m p) c -> p m c", m=NCH))
                nc.sync.dma_start(
                    out=gtC[:, b_], in_=gt[b_][:].rearrange("(m p) c -> p m c", m=NCH))
                nc.sync.dma_start(
                    out=maskC[:, b_],
                    in_=kmask[b_][:].rearrange("(c p) -> p c", p=128))

                # ---------------- pred2gt (chunk pairs) ----------------
                for mp in range(NCH // 2):
                    eP = mrg.tile([128, 2, NG], FP16, tag="eP")
                    keyP = mrg.tile([128, 2, NG], FP16, tag="keyP")
                    psNCs = []
                    for mi in range(2):
                        m = 2 * mp + mi
                        sl = slice(128 * m, 128 * (m + 1))
                        psZ = pszp.tile([128, NG], F32, tag="psZ")
                        nc.tensor.matmul(psZ[:], lhsT=lhsZ[:, sl], rhs=rhsZ[:],
                                         start=True, stop=True)
                        psNC = psncp.tile([128, NG], F32, tag="psNC")
                        nc.tensor.matmul(psNC[:], lhsT=lhsNC[:, sl],
                                         rhs=rhsNC[:], start=True, stop=True)
                        psNCs.append(psNC)
                        # t* = clamp(round(z),0,9): fp16 +1024 rounding trick
                        c1 = drp.tile([128, NG], FP16, tag="c1")
                        nc.scalar.activation(out=c1[:], in_=psZ[:], func=AF.Relu)
                        c2 = drp.tile([128, NG], FP16, tag="c2")
                        nc.scalar.activation(out=c2[:], in_=c1[:], func=AF.Relu,
                                             bias=c9[:, 0:1], scale=-1.0)
                        yv = drp.tile([128, NG], FP16, tag="yv")
                        nc.scalar.activation(out=yv[:], in_=c2[:], func=AF.Copy,
                                             bias=1024.0)
                        # e = z - t = z + y - 1033
                        nc.vector.scalar_tensor_tensor(
                            out=eP[:, mi], in0=yv[:], scalar=-1033.0,
                            in1=psZ[:], op0=ALU.add, op1=ALU.add)
                    # pairwise SBUF-only ops run double-width
                    mP = mrg.tile([128, 2, NG], FP16, tag="mP")
                    nc.vector.tensor_tensor(
                        out=mP[:], in0=eP[:],
                        in1=slf[:].unsqueeze(1).to_broadcast([128, 2, NG]),
                        op=ALU.mult)
                    m2P = mrg.tile([128, 2, NG], FP16, tag="m2P")
                    nc.vector.tensor_tensor(out=m2P[:], in0=mP[:], in1=mP[:],
                                            op=ALU.mult)
                    for mi in range(2):
                        nc.vector.scalar_tensor_tensor(
                            out=keyP[:, mi], in0=m2P[:, mi], scalar=-1.0,
                            in1=psNCs[mi][:], op0=ALU.mult, op1=ALU.add)
                    # segment-pair max on the (otherwise idle) gpsimd
                    # engine halves the DVE reduce+find width; the 20-point
                    # refine over BOTH segments of the winning pair absorbs
                    # the pair ambiguity exactly
                    pbP = mrg.tile([128, 2, NG // 2], FP16, tag="pbP")
                    for mi in range(2):
                        kv = keyP[:, mi].rearrange("p (n two) -> p n two",
                                                   two=2)
                        nc.vector.tensor_tensor(out=pbP[:, mi], in0=kv[:, :, 0],
                                                in1=kv[:, :, 1], op=ALU.max)
                    mxP = small.tile([128, 2], FP16, tag="mxP")
                    nc.vector.tensor_reduce(out=mxP[:], in_=pbP[:], axis=AX.X,
                                            op=ALU.max)
                    for mi in range(2):
                        m = 2 * mp + mi
                        i8 = small.tile([128, 8], U32, tag="i8")
                        nc.vector.max_index(
                            out=i8[:],
                            in_max=mxP[:, mi:mi + 1].to_broadcast([128, 8]),
                            in_values=pbP[:, mi])
                        nc.gpsimd.indirect_dma_start(
                            out=candC[:, b_, m].rearrange("p t c -> p (t c)"),
                            out_offset=None,
                            in_=itab_d[:].rearrange("(r two) w -> r (two w)",
                                                    two=2),
                            in_offset=IndirectOffsetOnAxis(ap=i8[:, 0:1],
                                                           axis=0),
                            element_offset=b_ * NG * T * 2)

                # ---------------- gt2pred ----------------
                for c in range(NCH):
                    sl = slice(128 * c, 128 * (c + 1))
                    ps2 = psgp.tile([128, NP], F32, tag="ps2")
                    nc.tensor.matmul(ps2[:], lhsT=gtl10[:, sl], rhs=prhs10[:],
                                     start=True, stop=True)
                    key2 = mrg.tile([128, NP], FP16, tag="key2")
                    nc.scalar.activation(out=key2[:], in_=ps2[:], func=AF.Copy,
                                         bias=0.0)
                    gmx = small.tile([128, 1], FP16, tag="gmx")
                    nc.vector.tensor_reduce(out=gmx[:], in_=key2[:], axis=AX.X,
                                            op=ALU.max)
                    ig8 = small.tile([128, 8], U32, tag="ig8")
                    nc.vector.max_index(out=ig8[:],
                                        in_max=gmx[:].to_broadcast([128, 8]),
                                        in_values=key2[:])
                    nc.gpsimd.indirect_dma_start(
                        out=npredC[:, b_, c, :], out_offset=None,
                        in_=ptab_d[:],
                        in_offset=IndirectOffsetOnAxis(ap=ig8[:, 0:1], axis=0),
                        element_offset=b_ * NP * 2)
                    if DEBUG and b_ == 0 and c == 0:
                        nc.sync.dma_start(out=dbg_pb[:], in_=key2[:])

            if DEBUG:
                nc.sync.dma_start(
                    out=dbg_candC[:],
                    in_=candC[:].rearrange("p b c t x -> p (b c t x)"))

            # ---------------- pred2gt refine + loss tail ----------------
            TC = 2 * T
            SH4 = [128, BLOC, NCH, TC]
            dx = small.tile([128, BLOC, NCH, TC], F32, tag="dx")
            dy = small.tile([128, BLOC, NCH, TC], F32, tag="dy")
            nc.vector.tensor_tensor(
                out=dx[:], in0=candC[:, :, :, :, 0],
                in1=pxyC[:, :, :, 0:1].to_broadcast(SH4), op=ALU.subtract)
            nc.vector.tensor_tensor(
                out=dy[:], in0=candC[:, :, :, :, 1],
                in1=pxyC[:, :, :, 1:2].to_broadcast(SH4), op=ALU.subtract)
            sqx = small.tile([128, BLOC, NCH, TC], F32, tag="sqx")
            sqy = small.tile([128, BLOC, NCH, TC], F32, tag="sqy")
            dall = small.tile([128, BLOC, NCH, TC], F32, tag="dall")
            nc.vector.tensor_tensor(out=sqx[:], in0=dx[:], in1=dx[:], op=ALU.mult)
            nc.vector.tensor_tensor(out=sqy[:], in0=dy[:], in1=dy[:], op=ALU.mult)
            nc.vector.tensor_tensor(out=dall[:], in0=sqx[:], in1=sqy[:], op=ALU.add)
            dmin = small.tile([128, BLOC, NCH], F32, tag="dmin")
            nc.vector.tensor_reduce(out=dmin[:], in_=dall[:], axis=AX.X, op=ALU.min)
            sel = small.tile([128, BLOC, NCH, TC], F32, tag="sel")
            nc.vector.tensor_tensor(
                out=sel[:], in0=dall[:],
                in1=dmin[:].unsqueeze(3).to_broadcast(SH4), op=ALU.is_equal)
            selx = small.tile([128, BLOC, NCH, TC], F32, tag="selx")
            sely = small.tile([128, BLOC, NCH, TC], F32, tag="sely")
            nc.vector.tensor_tensor(out=selx[:], in0=sel[:],
                                    in1=candC[:, :, :, :, 0], op=ALU.mult)
            nc.vector.tensor_tensor(out=sely[:], in0=sel[:],
                                    in1=candC[:, :, :, :, 1], op=ALU.mult)
            # normalize by tie count to stay exact when two grid points tie
            cnt = small.tile([128, BLOC, NCH], F32, tag="cnt")
            nc.vector.tensor_reduce(out=cnt[:], in_=sel[:], axis=AX.X, op=ALU.add)
            rcnt = small.tile([128, BLOC, NCH], F32, tag="rcnt")
            nc.vector.reciprocal(out=rcnt[:], in_=cnt[:])
            nxs = small.tile([128, BLOC, NCH], F32, tag="nxs")
            nys = small.tile([128, BLOC, NCH], F32, tag="nys")
            nc.vector.tensor_reduce(out=nxs[:], in_=selx[:], axis=AX.X, op=ALU.add)
            nc.vector.tensor_reduce(out=nys[:], in_=sely[:], axis=AX.X, op=ALU.add)
            nx = small.tile([128, BLOC, NCH], F32, tag="nx")
            ny = small.tile([128, BLOC, NCH], F32, tag="ny")
            nc.vector.tensor_tensor(out=nx[:], in0=nxs[:], in1=rcnt[:], op=ALU.mult)
            nc.vector.tensor_tensor(out=ny[:], in0=nys[:], in1=rcnt[:], op=ALU.mult)
            df = small.tile([128, BLOC, NCH, 2], F32, tag="df")
            nc.vector.tensor_tensor(out=df[:, :, :, 0], in0=pred2C[:, :, :, 0],
                                    in1=nx[:], op=ALU.subtract)
            nc.vector.tensor_tensor(out=df[:, :, :, 1], in0=pred2C[:, :, :, 1],
                                    in1=ny[:], op=ALU.subtract)
            nc.vector.tensor_reduce(out=res[:, 0:BLOC], in_=df[:], axis=AX.XY,
                                    op=ALU.add, apply_absolute_value=True)

            # ---------------- gt2pred loss tail ----------------
            md = small.tile([128, BLOC, NCH, 2], F32, tag="md")
            nc.vector.tensor_tensor(out=md[:], in0=npredC[:], in1=gtC[:],
                                    op=ALU.subtract)
            sabs = small.tile([128, BLOC, NCH], F32, tag="sabs")
            nc.vector.tensor_reduce(out=sabs[:], in_=md[:], axis=AX.X,
                                    op=ALU.add, apply_absolute_value=True)
            smask = small.tile([128, BLOC, NCH], F32, tag="smask")
            nc.vector.tensor_tensor(out=smask[:], in0=sabs[:], in1=maskC[:],
                                    op=ALU.mult)
            nc.vector.tensor_reduce(out=res[:, 4:4 + BLOC], in_=smask[:],
                                    axis=AX.X, op=ALU.add)
            nc.vector.tensor_reduce(out=res[:, 8:8 + BLOC], in_=maskC[:],
                                    axis=AX.X, op=ALU.add)

            nc.sync.dma_start(out=out[:], in_=res[:])

    nc.compile()
    return nc


_NC_CACHE = None


def _get_nc():
    global _NC_CACHE
    if _NC_CACHE is None:
        _NC_CACHE = build_nc()
    return _NC_CACHE


def make_in_maps(ini_pred_poly, pred_polys_, gt_polys, keyPointsMask):
    in_maps = []
    for i in range(NCORES):
        s = slice(BLOC * i, BLOC * (i + 1))
        ini = np.ascontiguousarray(ini_pred_poly[s], dtype=np.float32)
        p2 = np.ascontiguousarray(pred_polys_[s], dtype=np.float32)
        gp = np.ascontiguousarray(gt_polys[s], dtype=np.float32)
        km = np.ascontiguousarray(keyPointsMask[s], dtype=np.float32)
        hp = host_prep(ini, p2, gp)
        im = {
            "ini_pred_poly": ini,
            "pred_polys_": p2,
            "gt_polys": gp,
            "keyPointsMask": km,
        }
        im.update(hp)
        in_maps.append(im)
    return in_maps


def combine_outputs(outs):
    """outs: list of [128, 12] per-core partial sums -> scalar loss (float32)."""
    acc = np.zeros(12, dtype=np.float64)
    for o in outs:
        acc += o.astype(np.float64).sum(axis=0)
    s_p2g = acc[0:4].sum()          # sum |pred_polys_ - nearest_gt|
    s_g2p = acc[4:8].sum()          # sum mask * |nearest_pred - gt|
    s_msk = 2.0 * acc[8:12].sum()   # sum of broadcast mask
    loss_pred2gt = s_p2g / (B * NP * 2)
    loss = (s_g2p / (s_msk + 1.0) + loss_pred2gt) / 2.0
    return np.float32(loss)


def kernel(ini_pred_poly, pred_polys_, gt_polys, keyPointsMask):
    nc = _get_nc()
    in_maps = make_in_maps(ini_pred_poly, pred_polys_, gt_polys, keyPointsMask)
    r = run_bass_kernel_spmd(nc, in_maps, list(range(NCORES)))
    return combine_outputs([r.results[i]["out"] for i in range(NCORES)])


if __name__ == "__main__":
    import reference

    inputs = {k: np.asarray(v) for k, v in reference.setup_inputs().items()}
    got = kernel(**inputs)
    print("kernel loss:", got)
